# revision 7
# baseline (speedup 1.0000x reference)
"""HMM forward (CgpHmmCell) Trainium2 kernel, v4.

Design (8 cores, time-split 32 ways globally):
  - Host reformats the one-hot x into obs indices (lossless argmax of the
    0/1 input) and uploads int16 pair-codes; the device fetches emission
    columns with the transposing hardware gather (dma_gather
    transpose=True) from an HBM pair-table
        tab[mA*125+mB] = [128*Bm[mA] | 128*Bm[mB]]   (bf16, 256B rows),
    which lands E^T directly in SBUF state-major layout. This replaces
    the baseline's entire one-hot stream + PE transposes + emission
    matmuls + PSUM->SBUF copies.
  - Each core runs NSTACKS=2 independent "stacks"; a stack advances TWO
    time segments block-diagonally on the 128 partitions (v[0:64]=segA
    states, v[64:128]=segB, 512 columns = sequences), so one
    [128x128]@[128,512] bf16 matmul is the whole transition for both.
    32 segments x 128 owned steps tile t=[0,4096); W=8 warmup layers
    re-converge each segment's state (the recursion forgets its init).
  - v' = E^T (*) (A @ v): transition on PE, elementwise multiply split
    DVE/Pool by columns (ets in SBUF + u in PSUM satisfies the one-PSUM
    operand rule). The 128x table scale centers the per-layer mass drift
    near 2^0, so no mid-segment rescale is needed at all; ones-matmul
    probes at the four segment-boundary layers record per-sequence
    masses and the host sums log-mass deltas with exact scale
    corrections.
  - Emission gathers are 512-index single-packet transposed dma_gathers
    (the fast evt_accel path; >512 idx/packet crashes the exec unit)
    rotated over 4 SWDGE queues -- 0.58 ns/idx streamed vs 6-8 ns/idx
    for every other gather configuration measured on this hardware.

Self-contained: hardcodes shapes for the 512x4096x125/S=64 problem.
"""

import numpy as np

import concourse.bass as bass
import concourse.tile as tile
from concourse import bacc, mybir
from concourse import bass_utils

B, T, S, M = 512, 4096, 64, 125
NCORES = 8
NSTACKS = 2
SEGS = NCORES * NSTACKS * 2          # 32 global segments
SEG_T = T // SEGS                    # 128 owned steps per segment
W = 8                                # warmup layers
L = SEG_T + W + 1                    # 145 layers per stack
CH = 4                               # layers per gather chunk
TBL = M * M                          # 15625 pair-table rows
TBL_SCALE = 128.0                    # table scale, corrected on host

PROBE_LAYERS = [W, SEG_T, L - 2, L - 1]
PIDX = {l: r for r, l in enumerate(PROBE_LAYERS)}
NPROBE = len(PROBE_LAYERS)           # 4

F32 = mybir.dt.float32
BF16 = mybir.dt.bfloat16
I16 = mybir.dt.int16


def _build_program(reps=1, nstacks=NSTACKS, nlayers=L):
    nc = bacc.Bacc("TRN2", target_bir_lowering=False, debug=False,
                   num_devices=NCORES, dynamic_dma_scratch_size=131072,
                   num_swdge_queues=4)

    tab_d = nc.dram_tensor("tab", [TBL, 128], BF16, kind="ExternalInput")
    idx_d = nc.dram_tensor("idx", [nstacks, 128, nlayers * 32], I16,
                           kind="ExternalInput")
    a2_d = nc.dram_tensor("a2", [128, 128], BF16, kind="ExternalInput")
    icol_d = nc.dram_tensor("icol", [128, 1], F32, kind="ExternalInput")
    ones_d = nc.dram_tensor("ones", [128, 1], BF16, kind="ExternalInput")
    out_d = nc.dram_tensor("slots", [128, nstacks * 8 * NPROBE], F32,
                           kind="ExternalOutput")

    with tile.TileContext(nc) as tc:
        with (
            tc.tile_pool(name="const", bufs=1) as constp,
            tc.tile_pool(name="state", bufs=1) as statep,
            tc.tile_pool(name="idxp", bufs=3) as idxp,
            tc.tile_pool(name="ets", bufs=6) as etsp,
            tc.tile_pool(name="up", bufs=1, space="PSUM") as upp,
            tc.tile_pool(name="zp", bufs=2, space="PSUM") as zpp,
        ):
            a2 = constp.tile([128, 128], BF16)
            icol = constp.tile([128, 1], F32)
            ones = constp.tile([128, 1], BF16)
            nc.sync.dma_start(a2[:], a2_d.ap())
            nc.sync.dma_start(icol[:], icol_d.ap())
            nc.sync.dma_start(ones[:], ones_d.ap())

            v = [statep.tile([128, 512], BF16, name=f"v{st}")
                 for st in range(nstacks)]
            slots = [statep.tile([128, 8, NPROBE], F32, name=f"slots{st}")
                     for st in range(nstacks)]

            env = dict(nc=nc, tc=tc, nstacks=nstacks, nlayers=nlayers,
                       a2=a2, icol=icol, ones=ones, v=v, slots=slots,
                       idx_d=idx_d, tab_d=tab_d,
                       idxp=idxp, etsp=etsp, upp=upp, zpp=zpp)

            import contextlib
            loop_cm = (tc.For_i(0, reps, 1) if reps > 1
                       else contextlib.nullcontext())
            with loop_cm:
                _emit_body(env)

            for st in range(nstacks):
                nc.sync.dma_start(
                    out_d.ap()[:, st * 8 * NPROBE:(st + 1) * 8 * NPROBE],
                    slots[st][:].rearrange("p q r -> p (q r)"))

    nc.compile()
    return nc


def _emit_body(env):
    nc = env["nc"]
    nstacks, nlayers = env["nstacks"], env["nlayers"]
    a2, icol, ones = env["a2"], env["icol"], env["ones"]
    v, slots = env["v"], env["slots"]
    idx_d, tab_d = env["idx_d"], env["tab_d"]
    idxp, etsp, upp, zpp = env["idxp"], env["etsp"], env["upp"], env["zpp"]

    idxt = [None] * nstacks
    etc = [None] * nstacks
    nq = 0

    for l in range(nlayers):
        c, tt = divmod(l, CH)
        for st in range(nstacks):
            if tt == 0:
                ch = min(CH, nlayers - c * CH)
                idxc = idxp.tile([128, ch * 32], I16, name=f"idxc{st}",
                                 tag=f"idx{st}")
                nc.sync.dma_start(
                    idxc[:],
                    idx_d.ap()[st, :, c * CH * 32:c * CH * 32 + ch * 32])
                idxt[st] = idxc

            if tt == 0:
                ch = min(CH, nlayers - c * CH)
                etc[st] = etsp.tile([128, 1, ch * 512], BF16,
                                    name=f"ets{st}", tag=f"ets{st}")
                nc.gpsimd.dma_gather(
                    etc[st][:], tab_d.ap(), idxt[st][:],
                    num_idxs=ch * 512, num_idxs_reg=ch * 512, elem_size=128,
                    transpose=True, single_packet=False, queue_num=nq % 4)
                nq += 1

            etf = etc[st][:, 0, tt * 512:tt * 512 + 512]
            if l == 0:
                nc.vector.tensor_scalar_mul(v[st][:], etf, icol[:])
            else:
                u = upp.tile([128, 512], F32, name=f"u{st}", tag=f"u{st}")
                nc.tensor.matmul(out=u[:], lhsT=a2[:], rhs=v[st][:])
                nc.vector.tensor_mul(v[st][:], u[:], etf)

            if l in PIDX and nlayers == L:
                r = PIDX[l]
                zp = zpp.tile([128, 8], F32, name=f"zp{st}", tag=f"zp{st}")
                for h in range(2):
                    for g in range(4):
                        nc.tensor.matmul(
                            out=zp[:, 4 * h + g:4 * h + g + 1],
                            lhsT=v[st][64 * h:64 * h + 64,
                                       128 * g:128 * g + 128],
                            rhs=ones[64 * h:64 * h + 64, :])
                nc.vector.reciprocal(slots[st][:, :, r:r + 1], zp[:, :])


_NC_CACHE = None


def _get_program():
    global _NC_CACHE
    if _NC_CACHE is None:
        _NC_CACHE = _build_program()
    return _NC_CACHE


def _to_bf16(a):
    import ml_dtypes
    return np.asarray(a, np.float32).astype(ml_dtypes.bfloat16)


def _host_inputs(x, I, A, Bm, obs=None):
    """Per-core in_maps for run_bass_kernel_spmd."""
    if obs is None:
        obs = np.argmax(np.asarray(x), axis=2).astype(np.int64)  # [B, T]
    I = np.asarray(I, np.float32).reshape(S)
    A = np.asarray(A, np.float32)
    Bm = np.asarray(Bm, np.float32)

    a2 = np.zeros((128, 128), np.float32)
    a2[:S, :S] = A
    a2[S:, S:] = A
    a2 = _to_bf16(a2)

    BmS = (TBL_SCALE * Bm).astype(np.float32)            # [125, 64]
    tab = np.zeros((M, M, 128), np.float32)
    tab[:, :, 0:64] = BmS[:, None, :]
    tab[:, :, 64:128] = BmS[None, :, :]
    tab = _to_bf16(tab.reshape(TBL, 128))

    ones_b = _to_bf16(np.ones((128, 1), np.float32))

    def seg_ts(G):
        t0 = 0 if G == 0 else SEG_T * G - W
        return np.clip(np.arange(t0, t0 + L), 0, T - 1)

    in_maps = []
    for cidx in range(NCORES):
        idx = np.zeros((NSTACKS, 128, L * 32), np.int16)
        for st in range(NSTACKS):
            GA = 4 * cidx + 2 * st
            tsA, tsB = seg_ts(GA), seg_ts(GA + 1)
            codes = (obs[:, tsA] * M + obs[:, tsB]).astype(np.int16)  # [B, L]
            # unwrapped order i = l*512 + b; idx16[p, j] = unwrapped[j*16+p%16]
            unw = np.ascontiguousarray(codes.T).reshape(L * 512)
            wrap = unw.reshape(L * 32, 16).T                 # [16, L*32]
            idx[st] = np.tile(wrap, (8, 1))
        icol = np.ones((128, 1), np.float32)
        if cidx == 0:
            icol[0:64, 0] = I
        in_maps.append({
            "tab": tab,
            "idx": idx,
            "a2": a2,
            "icol": icol,
            "ones": ones_b,
        })
    return in_maps


def _host_reduce(results):
    """Combine per-core slot reciprocals into ll [B, 1] float32."""
    lnS = np.log(np.float64(TBL_SCALE))
    ll = np.zeros((B,), np.float64)
    for cidx in range(NCORES):
        sl = np.asarray(results[cidx]["slots"], np.float32).reshape(
            128, NSTACKS, 8, NPROBE).astype(np.float64)
        logm = -np.log(sl)                   # [p, st, q=(4h+g), r]
        for st in range(NSTACKS):
            for h in range(2):
                G = 4 * cidx + 2 * st + h
                lm = logm[:, st, 4 * h:4 * h + 4, :]     # [p, g, r]
                if G == 0:
                    contrib = lm[:, :, PIDX[SEG_T]] - (SEG_T + 1) * lnS
                elif G < SEGS - 1:
                    contrib = (lm[:, :, PIDX[L - 1]] - lm[:, :, PIDX[W]]
                               - SEG_T * lnS)
                else:
                    contrib = (lm[:, :, PIDX[L - 2]] - lm[:, :, PIDX[W]]
                               - (SEG_T - 1) * lnS)
                # sequence b = 128g + p
                ll += contrib.T.reshape(B)
    return ll.reshape(B, 1).astype(np.float32)


def kernel(x, I, A, Bm):
    nc = _get_program()
    in_maps = _host_inputs(x, I, A, Bm)
    res = bass_utils.run_bass_kernel_spmd(nc, in_maps,
                                          core_ids=list(range(NCORES)))
    return _host_reduce(res.results)


# revision 8
# speedup vs baseline: 1.0631x; 1.0631x over previous
"""HMM forward (CgpHmmCell) Trainium2 kernel, v4.

Design (8 cores, time-split 32 ways globally):
  - Host reformats the one-hot x into obs indices (lossless argmax of the
    0/1 input) and uploads int16 pair-codes; the device fetches emission
    columns with the transposing hardware gather (dma_gather
    transpose=True) from an HBM pair-table
        tab[mA*125+mB] = [128*Bm[mA] | 128*Bm[mB]]   (bf16, 256B rows),
    which lands E^T directly in SBUF state-major layout. This replaces
    the baseline's entire one-hot stream + PE transposes + emission
    matmuls + PSUM->SBUF copies.
  - Each core runs NSTACKS=2 independent "stacks"; a stack advances TWO
    time segments block-diagonally on the 128 partitions (v[0:64]=segA
    states, v[64:128]=segB, 512 columns = sequences), so one
    [128x128]@[128,512] bf16 matmul is the whole transition for both.
    32 segments x 128 owned steps tile t=[0,4096); W=8 warmup layers
    re-converge each segment's state (the recursion forgets its init).
  - v' = E^T (*) (A @ v): transition on PE, elementwise multiply split
    DVE/Pool by columns (ets in SBUF + u in PSUM satisfies the one-PSUM
    operand rule). The 128x table scale centers the per-layer mass drift
    near 2^0, so no mid-segment rescale is needed at all; ones-matmul
    probes at the four segment-boundary layers record per-sequence
    masses and the host sums log-mass deltas with exact scale
    corrections.
  - Emission gathers are 512-index single-packet transposed dma_gathers
    (the fast evt_accel path; >512 idx/packet crashes the exec unit)
    rotated over 4 SWDGE queues -- 0.58 ns/idx streamed vs 6-8 ns/idx
    for every other gather configuration measured on this hardware.

Self-contained: hardcodes shapes for the 512x4096x125/S=64 problem.
"""

import numpy as np

import concourse.bass as bass
import concourse.tile as tile
from concourse import bacc, mybir
from concourse import bass_utils

B, T, S, M = 512, 4096, 64, 125
NCORES = 8
NSTACKS = 2
SEGS = NCORES * NSTACKS * 2          # 32 global segments
SEG_T = T // SEGS                    # 128 owned steps per segment
W = 8                                # warmup layers
L = SEG_T + W + 1                    # 145 layers per stack
CH = 4                               # layers per gather chunk
TBL = M * M                          # 15625 pair-table rows
TBL_SCALE = 128.0                    # table scale, corrected on host

PROBE_LAYERS = [W, SEG_T, L - 2, L - 1]
PIDX = {l: r for r, l in enumerate(PROBE_LAYERS)}
NPROBE = len(PROBE_LAYERS)           # 4

F32 = mybir.dt.float32
BF16 = mybir.dt.bfloat16
I16 = mybir.dt.int16


def _build_program(reps=1, nstacks=NSTACKS, nlayers=L):
    nc = bacc.Bacc("TRN2", target_bir_lowering=False, debug=False,
                   num_devices=NCORES, dynamic_dma_scratch_size=131072,
                   num_swdge_queues=4)

    tab_d = nc.dram_tensor("tab", [TBL, 128], BF16, kind="ExternalInput")
    idx_d = nc.dram_tensor("idx", [nstacks, 128, nlayers * 32], I16,
                           kind="ExternalInput")
    a2_d = nc.dram_tensor("a2", [128, 128], BF16, kind="ExternalInput")
    icol_d = nc.dram_tensor("icol", [128, 1], F32, kind="ExternalInput")
    ones_d = nc.dram_tensor("ones", [128, 1], BF16, kind="ExternalInput")
    out_d = nc.dram_tensor("slots", [128, nstacks * 8 * NPROBE], F32,
                           kind="ExternalOutput")

    with tile.TileContext(nc) as tc:
        with (
            tc.tile_pool(name="const", bufs=1) as constp,
            tc.tile_pool(name="state", bufs=1) as statep,
            tc.tile_pool(name="idxp", bufs=3) as idxp,
            tc.tile_pool(name="ets", bufs=4) as etsp,
            tc.tile_pool(name="up", bufs=1, space="PSUM") as upp,
            tc.tile_pool(name="zp", bufs=2, space="PSUM") as zpp,
        ):
            a2 = constp.tile([128, 128], BF16)
            icol = constp.tile([128, 1], F32)
            ones = constp.tile([128, 1], BF16)
            nc.sync.dma_start(a2[:], a2_d.ap())
            nc.sync.dma_start(icol[:], icol_d.ap())
            nc.sync.dma_start(ones[:], ones_d.ap())

            v = [statep.tile([128, 512], BF16, name=f"v{st}")
                 for st in range(nstacks)]
            slots = [statep.tile([128, 8, NPROBE], F32, name=f"slots{st}")
                     for st in range(nstacks)]

            env = dict(nc=nc, tc=tc, nstacks=nstacks, nlayers=nlayers,
                       a2=a2, icol=icol, ones=ones, v=v, slots=slots,
                       idx_d=idx_d, tab_d=tab_d,
                       idxp=idxp, etsp=etsp, upp=upp, zpp=zpp)

            import contextlib
            loop_cm = (tc.For_i(0, reps, 1) if reps > 1
                       else contextlib.nullcontext())
            with loop_cm:
                _emit_body(env)

            for st in range(nstacks):
                nc.sync.dma_start(
                    out_d.ap()[:, st * 8 * NPROBE:(st + 1) * 8 * NPROBE],
                    slots[st][:].rearrange("p q r -> p (q r)"))

    nc.compile()
    return nc


def _emit_body(env):
    nc = env["nc"]
    nstacks, nlayers = env["nstacks"], env["nlayers"]
    a2, icol, ones = env["a2"], env["icol"], env["ones"]
    v, slots = env["v"], env["slots"]
    idx_d, tab_d = env["idx_d"], env["tab_d"]
    idxp, etsp, upp, zpp = env["idxp"], env["etsp"], env["upp"], env["zpp"]

    idxt = [None] * nstacks
    etc = [None] * nstacks
    nq = 0

    for l in range(nlayers):
        c, tt = divmod(l, CH)
        for st in range(nstacks):
            if tt == 0:
                ch = min(CH, nlayers - c * CH)
                idxc = idxp.tile([128, ch * 32], I16, name=f"idxc{st}",
                                 tag=f"idx{st}")
                nc.sync.dma_start(
                    idxc[:],
                    idx_d.ap()[st, :, c * CH * 32:c * CH * 32 + ch * 32])
                idxt[st] = idxc

            if tt == 0:
                ch = min(CH, nlayers - c * CH)
                etc[st] = etsp.tile([128, 1, ch * 512], BF16,
                                    name=f"ets{st}", tag=f"ets{st}")
                nc.gpsimd.dma_gather(
                    etc[st][:], tab_d.ap(), idxt[st][:],
                    num_idxs=ch * 512, num_idxs_reg=ch * 512, elem_size=128,
                    transpose=True, single_packet=False, queue_num=nq % 4)
                nq += 1

            etf = etc[st][:, 0, tt * 512:tt * 512 + 512]
            if l == 0:
                nc.vector.tensor_scalar_mul(v[st][:], etf, icol[:])
            else:
                u = upp.tile([128, 512], F32, name=f"u{st}", tag=f"u{st}")
                nc.tensor.matmul(out=u[:], lhsT=a2[:], rhs=v[st][:])
                nc.vector.tensor_mul(v[st][:], u[:], etf)

            if l in PIDX and nlayers == L:
                r = PIDX[l]
                zp = zpp.tile([128, 8], F32, name=f"zp{st}", tag=f"zp{st}")
                for h in range(2):
                    for g in range(4):
                        nc.tensor.matmul(
                            out=zp[:, 4 * h + g:4 * h + g + 1],
                            lhsT=v[st][64 * h:64 * h + 64,
                                       128 * g:128 * g + 128],
                            rhs=ones[64 * h:64 * h + 64, :])
                nc.vector.reciprocal(slots[st][:, :, r:r + 1], zp[:, :])


_NC_CACHE = None


def _get_program():
    global _NC_CACHE
    if _NC_CACHE is None:
        _NC_CACHE = _build_program()
    return _NC_CACHE


def _to_bf16(a):
    import ml_dtypes
    return np.asarray(a, np.float32).astype(ml_dtypes.bfloat16)


def _host_inputs(x, I, A, Bm, obs=None):
    """Per-core in_maps for run_bass_kernel_spmd."""
    if obs is None:
        obs = np.argmax(np.asarray(x), axis=2).astype(np.int64)  # [B, T]
    I = np.asarray(I, np.float32).reshape(S)
    A = np.asarray(A, np.float32)
    Bm = np.asarray(Bm, np.float32)

    a2 = np.zeros((128, 128), np.float32)
    a2[:S, :S] = A
    a2[S:, S:] = A
    a2 = _to_bf16(a2)

    BmS = (TBL_SCALE * Bm).astype(np.float32)            # [125, 64]
    tab = np.zeros((M, M, 128), np.float32)
    tab[:, :, 0:64] = BmS[:, None, :]
    tab[:, :, 64:128] = BmS[None, :, :]
    tab = _to_bf16(tab.reshape(TBL, 128))

    ones_b = _to_bf16(np.ones((128, 1), np.float32))

    def seg_ts(G):
        t0 = 0 if G == 0 else SEG_T * G - W
        return np.clip(np.arange(t0, t0 + L), 0, T - 1)

    in_maps = []
    for cidx in range(NCORES):
        idx = np.zeros((NSTACKS, 128, L * 32), np.int16)
        for st in range(NSTACKS):
            GA = 4 * cidx + 2 * st
            tsA, tsB = seg_ts(GA), seg_ts(GA + 1)
            codes = (obs[:, tsA] * M + obs[:, tsB]).astype(np.int16)  # [B, L]
            # unwrapped order i = l*512 + b; idx16[p, j] = unwrapped[j*16+p%16]
            unw = np.ascontiguousarray(codes.T).reshape(L * 512)
            wrap = unw.reshape(L * 32, 16).T                 # [16, L*32]
            idx[st] = np.tile(wrap, (8, 1))
        icol = np.ones((128, 1), np.float32)
        if cidx == 0:
            icol[0:64, 0] = I
        in_maps.append({
            "tab": tab,
            "idx": idx,
            "a2": a2,
            "icol": icol,
            "ones": ones_b,
        })
    return in_maps


def _host_reduce(results):
    """Combine per-core slot reciprocals into ll [B, 1] float32."""
    lnS = np.log(np.float64(TBL_SCALE))
    ll = np.zeros((B,), np.float64)
    for cidx in range(NCORES):
        sl = np.asarray(results[cidx]["slots"], np.float32).reshape(
            128, NSTACKS, 8, NPROBE).astype(np.float64)
        logm = -np.log(sl)                   # [p, st, q=(4h+g), r]
        for st in range(NSTACKS):
            for h in range(2):
                G = 4 * cidx + 2 * st + h
                lm = logm[:, st, 4 * h:4 * h + 4, :]     # [p, g, r]
                if G == 0:
                    contrib = lm[:, :, PIDX[SEG_T]] - (SEG_T + 1) * lnS
                elif G < SEGS - 1:
                    contrib = (lm[:, :, PIDX[L - 1]] - lm[:, :, PIDX[W]]
                               - SEG_T * lnS)
                else:
                    contrib = (lm[:, :, PIDX[L - 2]] - lm[:, :, PIDX[W]]
                               - (SEG_T - 1) * lnS)
                # sequence b = 128g + p
                ll += contrib.T.reshape(B)
    return ll.reshape(B, 1).astype(np.float32)


def kernel(x, I, A, Bm):
    nc = _get_program()
    in_maps = _host_inputs(x, I, A, Bm)
    res = bass_utils.run_bass_kernel_spmd(nc, in_maps,
                                          core_ids=list(range(NCORES)))
    return _host_reduce(res.results)


# revision 9
# speedup vs baseline: 1.0661x; 1.0029x over previous
"""HMM forward (CgpHmmCell) Trainium2 kernel, v4.

Design (8 cores, time-split 32 ways globally):
  - Host reformats the one-hot x into obs indices (lossless argmax of the
    0/1 input) and uploads int16 pair-codes; the device fetches emission
    columns with the transposing hardware gather (dma_gather
    transpose=True) from an HBM pair-table
        tab[mA*125+mB] = [128*Bm[mA] | 128*Bm[mB]]   (bf16, 256B rows),
    which lands E^T directly in SBUF state-major layout. This replaces
    the baseline's entire one-hot stream + PE transposes + emission
    matmuls + PSUM->SBUF copies.
  - Each core runs NSTACKS=2 independent "stacks"; a stack advances TWO
    time segments block-diagonally on the 128 partitions (v[0:64]=segA
    states, v[64:128]=segB, 512 columns = sequences), so one
    [128x128]@[128,512] bf16 matmul is the whole transition for both.
    32 segments x 128 owned steps tile t=[0,4096); W=8 warmup layers
    re-converge each segment's state (the recursion forgets its init).
  - v' = E^T (*) (A @ v): transition on PE, elementwise multiply split
    DVE/Pool by columns (ets in SBUF + u in PSUM satisfies the one-PSUM
    operand rule). The 128x table scale centers the per-layer mass drift
    near 2^0, so no mid-segment rescale is needed at all; ones-matmul
    probes at the four segment-boundary layers record per-sequence
    masses and the host sums log-mass deltas with exact scale
    corrections.
  - Emission gathers are 512-index single-packet transposed dma_gathers
    (the fast evt_accel path; >512 idx/packet crashes the exec unit)
    rotated over 4 SWDGE queues -- 0.58 ns/idx streamed vs 6-8 ns/idx
    for every other gather configuration measured on this hardware.

Self-contained: hardcodes shapes for the 512x4096x125/S=64 problem.
"""

import numpy as np

import concourse.bass as bass
import concourse.tile as tile
from concourse import bacc, mybir
from concourse import bass_utils

B, T, S, M = 512, 4096, 64, 125
NCORES = 8
NSTACKS = 2
SEGS = NCORES * NSTACKS * 2          # 32 global segments
SEG_T = T // SEGS                    # 128 owned steps per segment
W = 8                                # warmup layers
L = SEG_T + W + 1                    # 145 layers per stack
CH = 4                               # layers per gather chunk
TBL = M * M                          # 15625 pair-table rows
TBL_SCALE = 128.0                    # table scale, corrected on host

PROBE_LAYERS = [W, SEG_T, L - 2, L - 1]
PIDX = {l: r for r, l in enumerate(PROBE_LAYERS)}
NPROBE = len(PROBE_LAYERS)           # 4

F32 = mybir.dt.float32
BF16 = mybir.dt.bfloat16
I16 = mybir.dt.int16


def _build_program(reps=1, nstacks=NSTACKS, nlayers=L):
    nc = bacc.Bacc("TRN2", target_bir_lowering=False, debug=False,
                   num_devices=NCORES, dynamic_dma_scratch_size=131072,
                   num_swdge_queues=4)

    tab_d = nc.dram_tensor("tab", [TBL, 128], BF16, kind="ExternalInput")
    idx_d = nc.dram_tensor("idx", [nstacks, 128, nlayers * 32], I16,
                           kind="ExternalInput")
    a2_d = nc.dram_tensor("a2", [128, 128], BF16, kind="ExternalInput")
    icol_d = nc.dram_tensor("icol", [128, 1], F32, kind="ExternalInput")
    ones_d = nc.dram_tensor("ones", [128, 1], BF16, kind="ExternalInput")
    out_d = nc.dram_tensor("slots", [128, nstacks * 8 * NPROBE], F32,
                           kind="ExternalOutput")

    with tile.TileContext(nc) as tc:
        with (
            tc.tile_pool(name="const", bufs=1) as constp,
            tc.tile_pool(name="state", bufs=1) as statep,
            tc.tile_pool(name="idxp", bufs=3) as idxp,
            tc.tile_pool(name="ets", bufs=4) as etsp,
            tc.tile_pool(name="up", bufs=1, space="PSUM") as upp,
            tc.tile_pool(name="zp", bufs=2, space="PSUM") as zpp,
        ):
            a2 = constp.tile([128, 128], BF16)
            icol = constp.tile([128, 1], F32)
            ones = constp.tile([128, 1], BF16)
            nc.sync.dma_start(a2[:], a2_d.ap())
            nc.sync.dma_start(icol[:], icol_d.ap())
            nc.sync.dma_start(ones[:], ones_d.ap())

            v = [statep.tile([128, 512], BF16, name=f"v{st}")
                 for st in range(nstacks)]
            slots = [statep.tile([128, 8, NPROBE], F32, name=f"slots{st}")
                     for st in range(nstacks)]

            env = dict(nc=nc, tc=tc, nstacks=nstacks, nlayers=nlayers,
                       a2=a2, icol=icol, ones=ones, v=v, slots=slots,
                       idx_d=idx_d, tab_d=tab_d,
                       idxp=idxp, etsp=etsp, upp=upp, zpp=zpp)

            import contextlib
            loop_cm = (tc.For_i(0, reps, 1) if reps > 1
                       else contextlib.nullcontext())
            with loop_cm:
                _emit_body(env)

            for st in range(nstacks):
                nc.sync.dma_start(
                    out_d.ap()[:, st * 8 * NPROBE:(st + 1) * 8 * NPROBE],
                    slots[st][:].rearrange("p q r -> p (q r)"))

    nc.compile()
    return nc


def _emit_body(env):
    nc = env["nc"]
    nstacks, nlayers = env["nstacks"], env["nlayers"]
    a2, icol, ones = env["a2"], env["icol"], env["ones"]
    v, slots = env["v"], env["slots"]
    idx_d, tab_d = env["idx_d"], env["tab_d"]
    idxp, etsp, upp, zpp = env["idxp"], env["etsp"], env["upp"], env["zpp"]

    etc = [None] * nstacks
    nq = 0

    def fetch(st, c):
        nonlocal nq
        if c * CH >= nlayers:
            return
        ch = min(CH, nlayers - c * CH)
        idxc = idxp.tile([128, ch * 32], I16, name=f"idxc{st}",
                         tag=f"idx{st}")
        nc.sync.dma_start(
            idxc[:],
            idx_d.ap()[st, :, c * CH * 32:c * CH * 32 + ch * 32])
        et = etsp.tile([128, 1, ch * 512], BF16,
                       name=f"ets{st}", tag=f"ets{st}")
        nc.gpsimd.dma_gather(
            et[:], tab_d.ap(), idxc[:],
            num_idxs=ch * 512, num_idxs_reg=ch * 512, elem_size=128,
            transpose=True, single_packet=False, queue_num=nq % 4)
        nq += 1
        etc[st] = et

    ecur = [None] * nstacks
    fetch(0, 0)
    fetch(1, 0)

    for l in range(nlayers):
        c, tt = divmod(l, CH)
        # staggered chunk prefetch: stack 0 fetches chunk c+1 at tt==0,
        # stack 1 at tt==2, so Pool preps and DMA bursts interleave
        if tt == 0:
            for st in range(nstacks):
                ecur[st] = etc[st]
            fetch(0, c + 1)
        if tt == 2:
            fetch(1, c + 1)
        for st in range(nstacks):
            etf = ecur[st][:, 0, tt * 512:tt * 512 + 512]
            if l == 0:
                nc.vector.tensor_scalar_mul(v[st][:], etf, icol[:])
            else:
                u = upp.tile([128, 512], F32, name=f"u{st}", tag=f"u{st}")
                nc.tensor.matmul(out=u[:], lhsT=a2[:], rhs=v[st][:])
                nc.vector.tensor_mul(v[st][:], u[:], etf)

            if l in PIDX and nlayers == L:
                r = PIDX[l]
                zp = zpp.tile([128, 8], F32, name=f"zp{st}", tag=f"zp{st}")
                for h in range(2):
                    for g in range(4):
                        nc.tensor.matmul(
                            out=zp[:, 4 * h + g:4 * h + g + 1],
                            lhsT=v[st][64 * h:64 * h + 64,
                                       128 * g:128 * g + 128],
                            rhs=ones[64 * h:64 * h + 64, :])
                nc.vector.reciprocal(slots[st][:, :, r:r + 1], zp[:, :])


_NC_CACHE = None


def _get_program():
    global _NC_CACHE
    if _NC_CACHE is None:
        _NC_CACHE = _build_program()
    return _NC_CACHE


def _to_bf16(a):
    import ml_dtypes
    return np.asarray(a, np.float32).astype(ml_dtypes.bfloat16)


def _host_inputs(x, I, A, Bm, obs=None):
    """Per-core in_maps for run_bass_kernel_spmd."""
    if obs is None:
        obs = np.argmax(np.asarray(x), axis=2).astype(np.int64)  # [B, T]
    I = np.asarray(I, np.float32).reshape(S)
    A = np.asarray(A, np.float32)
    Bm = np.asarray(Bm, np.float32)

    a2 = np.zeros((128, 128), np.float32)
    a2[:S, :S] = A
    a2[S:, S:] = A
    a2 = _to_bf16(a2)

    BmS = (TBL_SCALE * Bm).astype(np.float32)            # [125, 64]
    tab = np.zeros((M, M, 128), np.float32)
    tab[:, :, 0:64] = BmS[:, None, :]
    tab[:, :, 64:128] = BmS[None, :, :]
    tab = _to_bf16(tab.reshape(TBL, 128))

    ones_b = _to_bf16(np.ones((128, 1), np.float32))

    def seg_ts(G):
        t0 = 0 if G == 0 else SEG_T * G - W
        return np.clip(np.arange(t0, t0 + L), 0, T - 1)

    in_maps = []
    for cidx in range(NCORES):
        idx = np.zeros((NSTACKS, 128, L * 32), np.int16)
        for st in range(NSTACKS):
            GA = 4 * cidx + 2 * st
            tsA, tsB = seg_ts(GA), seg_ts(GA + 1)
            codes = (obs[:, tsA] * M + obs[:, tsB]).astype(np.int16)  # [B, L]
            # unwrapped order i = l*512 + b; idx16[p, j] = unwrapped[j*16+p%16]
            unw = np.ascontiguousarray(codes.T).reshape(L * 512)
            wrap = unw.reshape(L * 32, 16).T                 # [16, L*32]
            idx[st] = np.tile(wrap, (8, 1))
        icol = np.ones((128, 1), np.float32)
        if cidx == 0:
            icol[0:64, 0] = I
        in_maps.append({
            "tab": tab,
            "idx": idx,
            "a2": a2,
            "icol": icol,
            "ones": ones_b,
        })
    return in_maps


def _host_reduce(results):
    """Combine per-core slot reciprocals into ll [B, 1] float32."""
    lnS = np.log(np.float64(TBL_SCALE))
    ll = np.zeros((B,), np.float64)
    for cidx in range(NCORES):
        sl = np.asarray(results[cidx]["slots"], np.float32).reshape(
            128, NSTACKS, 8, NPROBE).astype(np.float64)
        logm = -np.log(sl)                   # [p, st, q=(4h+g), r]
        for st in range(NSTACKS):
            for h in range(2):
                G = 4 * cidx + 2 * st + h
                lm = logm[:, st, 4 * h:4 * h + 4, :]     # [p, g, r]
                if G == 0:
                    contrib = lm[:, :, PIDX[SEG_T]] - (SEG_T + 1) * lnS
                elif G < SEGS - 1:
                    contrib = (lm[:, :, PIDX[L - 1]] - lm[:, :, PIDX[W]]
                               - SEG_T * lnS)
                else:
                    contrib = (lm[:, :, PIDX[L - 2]] - lm[:, :, PIDX[W]]
                               - (SEG_T - 1) * lnS)
                # sequence b = 128g + p
                ll += contrib.T.reshape(B)
    return ll.reshape(B, 1).astype(np.float32)


def kernel(x, I, A, Bm):
    nc = _get_program()
    in_maps = _host_inputs(x, I, A, Bm)
    res = bass_utils.run_bass_kernel_spmd(nc, in_maps,
                                          core_ids=list(range(NCORES)))
    return _host_reduce(res.results)


# revision 10
# speedup vs baseline: 1.0745x; 1.0079x over previous
"""HMM forward (CgpHmmCell) Trainium2 kernel, v4.

Design (8 cores, time-split 32 ways globally):
  - Host reformats the one-hot x into obs indices (lossless argmax of the
    0/1 input) and uploads int16 pair-codes; the device fetches emission
    columns with the transposing hardware gather (dma_gather
    transpose=True) from an HBM pair-table
        tab[mA*125+mB] = [128*Bm[mA] | 128*Bm[mB]]   (bf16, 256B rows),
    which lands E^T directly in SBUF state-major layout. This replaces
    the baseline's entire one-hot stream + PE transposes + emission
    matmuls + PSUM->SBUF copies.
  - Each core runs NSTACKS=2 independent "stacks"; a stack advances TWO
    time segments block-diagonally on the 128 partitions (v[0:64]=segA
    states, v[64:128]=segB, 512 columns = sequences), so one
    [128x128]@[128,512] bf16 matmul is the whole transition for both.
    32 segments x 128 owned steps tile t=[0,4096); W=8 warmup layers
    re-converge each segment's state (the recursion forgets its init).
  - v' = E^T (*) (A @ v): transition on PE, elementwise multiply split
    DVE/Pool by columns (ets in SBUF + u in PSUM satisfies the one-PSUM
    operand rule). The 128x table scale centers the per-layer mass drift
    near 2^0, so no mid-segment rescale is needed at all; ones-matmul
    probes at the four segment-boundary layers record per-sequence
    masses and the host sums log-mass deltas with exact scale
    corrections.
  - Emission gathers are 512-index single-packet transposed dma_gathers
    (the fast evt_accel path; >512 idx/packet crashes the exec unit)
    rotated over 4 SWDGE queues -- 0.58 ns/idx streamed vs 6-8 ns/idx
    for every other gather configuration measured on this hardware.

Self-contained: hardcodes shapes for the 512x4096x125/S=64 problem.
"""

import numpy as np

import concourse.bass as bass
import concourse.tile as tile
from concourse import bacc, mybir
from concourse import bass_utils

B, T, S, M = 512, 4096, 64, 125
NCORES = 8
NSTACKS = 2
SEGS = NCORES * NSTACKS * 2          # 32 global segments
SEG_T = T // SEGS                    # 128 owned steps per segment
W = 8                                # warmup layers
L = SEG_T + W + 1                    # 145 layers per stack
CH = 5                               # layers per gather chunk
TBL = M * M                          # 15625 pair-table rows
TBL_SCALE = 128.0                    # table scale, corrected on host

PROBE_LAYERS = [W, SEG_T, L - 2, L - 1]
PIDX = {l: r for r, l in enumerate(PROBE_LAYERS)}
NPROBE = len(PROBE_LAYERS)           # 4

F32 = mybir.dt.float32
BF16 = mybir.dt.bfloat16
I16 = mybir.dt.int16


def _build_program(reps=1, nstacks=NSTACKS, nlayers=L):
    nc = bacc.Bacc("TRN2", target_bir_lowering=False, debug=False,
                   num_devices=NCORES, dynamic_dma_scratch_size=131072,
                   num_swdge_queues=4)

    tab_d = nc.dram_tensor("tab", [TBL, 128], BF16, kind="ExternalInput")
    idx_d = nc.dram_tensor("idx", [nstacks, 128, nlayers * 32], I16,
                           kind="ExternalInput")
    a2_d = nc.dram_tensor("a2", [128, 128], BF16, kind="ExternalInput")
    icol_d = nc.dram_tensor("icol", [128, 1], F32, kind="ExternalInput")
    ones_d = nc.dram_tensor("ones", [128, 1], BF16, kind="ExternalInput")
    out_d = nc.dram_tensor("slots", [128, nstacks * 8 * NPROBE], F32,
                           kind="ExternalOutput")

    with tile.TileContext(nc) as tc:
        with (
            tc.tile_pool(name="const", bufs=1) as constp,
            tc.tile_pool(name="state", bufs=1) as statep,
            tc.tile_pool(name="idxp", bufs=3) as idxp,
            tc.tile_pool(name="ets", bufs=4) as etsp,
            tc.tile_pool(name="up", bufs=1, space="PSUM") as upp,
            tc.tile_pool(name="zp", bufs=2, space="PSUM") as zpp,
        ):
            a2 = constp.tile([128, 128], BF16)
            icol = constp.tile([128, 1], F32)
            ones = constp.tile([128, 1], BF16)
            nc.sync.dma_start(a2[:], a2_d.ap())
            nc.sync.dma_start(icol[:], icol_d.ap())
            nc.sync.dma_start(ones[:], ones_d.ap())

            v = [statep.tile([128, 512], BF16, name=f"v{st}")
                 for st in range(nstacks)]
            slots = [statep.tile([128, 8, NPROBE], F32, name=f"slots{st}")
                     for st in range(nstacks)]

            env = dict(nc=nc, tc=tc, nstacks=nstacks, nlayers=nlayers,
                       a2=a2, icol=icol, ones=ones, v=v, slots=slots,
                       idx_d=idx_d, tab_d=tab_d,
                       idxp=idxp, etsp=etsp, upp=upp, zpp=zpp)

            import contextlib
            loop_cm = (tc.For_i(0, reps, 1) if reps > 1
                       else contextlib.nullcontext())
            with loop_cm:
                _emit_body(env)

            for st in range(nstacks):
                nc.sync.dma_start(
                    out_d.ap()[:, st * 8 * NPROBE:(st + 1) * 8 * NPROBE],
                    slots[st][:].rearrange("p q r -> p (q r)"))

    nc.compile()
    return nc


def _emit_body(env):
    nc = env["nc"]
    nstacks, nlayers = env["nstacks"], env["nlayers"]
    a2, icol, ones = env["a2"], env["icol"], env["ones"]
    v, slots = env["v"], env["slots"]
    idx_d, tab_d = env["idx_d"], env["tab_d"]
    idxp, etsp, upp, zpp = env["idxp"], env["etsp"], env["upp"], env["zpp"]

    etc = [None] * nstacks
    nq = 0

    def fetch(st, c):
        nonlocal nq
        if c * CH >= nlayers:
            return
        ch = min(CH, nlayers - c * CH)
        idxc = idxp.tile([128, ch * 32], I16, name=f"idxc{st}",
                         tag=f"idx{st}")
        nc.sync.dma_start(
            idxc[:],
            idx_d.ap()[st, :, c * CH * 32:c * CH * 32 + ch * 32])
        et = etsp.tile([128, 1, ch * 512], BF16,
                       name=f"ets{st}", tag=f"ets{st}")
        nc.gpsimd.dma_gather(
            et[:], tab_d.ap(), idxc[:],
            num_idxs=ch * 512, num_idxs_reg=ch * 512, elem_size=128,
            transpose=True, single_packet=False, queue_num=nq % 4)
        nq += 1
        etc[st] = et

    ecur = [None] * nstacks
    fetch(0, 0)
    fetch(1, 0)

    for l in range(nlayers):
        c, tt = divmod(l, CH)
        # staggered chunk prefetch: stack 0 fetches chunk c+1 at tt==0,
        # stack 1 at tt==2, so Pool preps and DMA bursts interleave
        if tt == 0:
            for st in range(nstacks):
                ecur[st] = etc[st]
            fetch(0, c + 1)
        if tt == 2:
            fetch(1, c + 1)
        for st in range(nstacks):
            etf = ecur[st][:, 0, tt * 512:tt * 512 + 512]
            if l == 0:
                nc.vector.tensor_scalar_mul(v[st][:], etf, icol[:])
            else:
                u = upp.tile([128, 512], F32, name=f"u{st}", tag=f"u{st}")
                nc.tensor.matmul(out=u[:], lhsT=a2[:], rhs=v[st][:])
                nc.vector.tensor_mul(v[st][:], u[:], etf)

            if l in PIDX and nlayers == L:
                r = PIDX[l]
                zp = zpp.tile([128, 8], F32, name=f"zp{st}", tag=f"zp{st}")
                for h in range(2):
                    for g in range(4):
                        nc.tensor.matmul(
                            out=zp[:, 4 * h + g:4 * h + g + 1],
                            lhsT=v[st][64 * h:64 * h + 64,
                                       128 * g:128 * g + 128],
                            rhs=ones[64 * h:64 * h + 64, :])
                nc.vector.reciprocal(slots[st][:, :, r:r + 1], zp[:, :])


_NC_CACHE = None


def _get_program():
    global _NC_CACHE
    if _NC_CACHE is None:
        _NC_CACHE = _build_program()
    return _NC_CACHE


def _to_bf16(a):
    import ml_dtypes
    return np.asarray(a, np.float32).astype(ml_dtypes.bfloat16)


def _host_inputs(x, I, A, Bm, obs=None):
    """Per-core in_maps for run_bass_kernel_spmd."""
    if obs is None:
        obs = np.argmax(np.asarray(x), axis=2).astype(np.int64)  # [B, T]
    I = np.asarray(I, np.float32).reshape(S)
    A = np.asarray(A, np.float32)
    Bm = np.asarray(Bm, np.float32)

    a2 = np.zeros((128, 128), np.float32)
    a2[:S, :S] = A
    a2[S:, S:] = A
    a2 = _to_bf16(a2)

    BmS = (TBL_SCALE * Bm).astype(np.float32)            # [125, 64]
    tab = np.zeros((M, M, 128), np.float32)
    tab[:, :, 0:64] = BmS[:, None, :]
    tab[:, :, 64:128] = BmS[None, :, :]
    tab = _to_bf16(tab.reshape(TBL, 128))

    ones_b = _to_bf16(np.ones((128, 1), np.float32))

    def seg_ts(G):
        t0 = 0 if G == 0 else SEG_T * G - W
        return np.clip(np.arange(t0, t0 + L), 0, T - 1)

    in_maps = []
    for cidx in range(NCORES):
        idx = np.zeros((NSTACKS, 128, L * 32), np.int16)
        for st in range(NSTACKS):
            GA = 4 * cidx + 2 * st
            tsA, tsB = seg_ts(GA), seg_ts(GA + 1)
            codes = (obs[:, tsA] * M + obs[:, tsB]).astype(np.int16)  # [B, L]
            # unwrapped order i = l*512 + b; idx16[p, j] = unwrapped[j*16+p%16]
            unw = np.ascontiguousarray(codes.T).reshape(L * 512)
            wrap = unw.reshape(L * 32, 16).T                 # [16, L*32]
            idx[st] = np.tile(wrap, (8, 1))
        icol = np.ones((128, 1), np.float32)
        if cidx == 0:
            icol[0:64, 0] = I
        in_maps.append({
            "tab": tab,
            "idx": idx,
            "a2": a2,
            "icol": icol,
            "ones": ones_b,
        })
    return in_maps


def _host_reduce(results):
    """Combine per-core slot reciprocals into ll [B, 1] float32."""
    lnS = np.log(np.float64(TBL_SCALE))
    ll = np.zeros((B,), np.float64)
    for cidx in range(NCORES):
        sl = np.asarray(results[cidx]["slots"], np.float32).reshape(
            128, NSTACKS, 8, NPROBE).astype(np.float64)
        logm = -np.log(sl)                   # [p, st, q=(4h+g), r]
        for st in range(NSTACKS):
            for h in range(2):
                G = 4 * cidx + 2 * st + h
                lm = logm[:, st, 4 * h:4 * h + 4, :]     # [p, g, r]
                if G == 0:
                    contrib = lm[:, :, PIDX[SEG_T]] - (SEG_T + 1) * lnS
                elif G < SEGS - 1:
                    contrib = (lm[:, :, PIDX[L - 1]] - lm[:, :, PIDX[W]]
                               - SEG_T * lnS)
                else:
                    contrib = (lm[:, :, PIDX[L - 2]] - lm[:, :, PIDX[W]]
                               - (SEG_T - 1) * lnS)
                # sequence b = 128g + p
                ll += contrib.T.reshape(B)
    return ll.reshape(B, 1).astype(np.float32)


def kernel(x, I, A, Bm):
    nc = _get_program()
    in_maps = _host_inputs(x, I, A, Bm)
    res = bass_utils.run_bass_kernel_spmd(nc, in_maps,
                                          core_ids=list(range(NCORES)))
    return _host_reduce(res.results)
